# revision 45
# baseline (speedup 1.0000x reference)
"""Multi-head attention (B=4, N=2048, D=768, H=12, Dh=64) on 8 TRN2 NeuronCores.

Sharding: core c -> batch b = c//2, head half hh = c%2 (6 heads each).
Each core computes its 6 heads over ALL 2048 queries x 2048 keys, so QKV
projection work is perfectly split (no duplicated K/V projection).  The final
projection is row-parallel over w_out: each core emits a bf16 partial
out = AO_local @ w_out[384 rows] and the host sums the two partials + bias.

Per-core graph (bf16 matmuls, f32 accumulation):
  1. Q^T/K^T pair panels [128=2x64 dims, 2048] projected per head pair.
  2. Scores per pair run ROW-TILED (64x128 PE mode): the two heads are
     independent K=64 matmuls on array row groups 0-63 / 64-127 executing
     concurrently -> 2x score throughput.  psS [128 keys, 1024] holds
     h0|h1 halves for one 512-query block.
  3. exp: 10 of 16 k-tiles exact Exp on ScalarE; 6 k-tiles a one-instruction
     Schraudolph approximation on VectorE (int16(23.083*s + 16250) bitcast
     to bf16 ~= exp(0.125 s), max rel err ~3.5%) -> both engines share the
     25M-element softmax which would otherwise bottleneck on ACT
     (~190us at 1 elem/cycle/lane).  End-to-end rel err ~9.2e-3 (gate 2e-2).
  4. attn@V per head: V stationary [128 keys, 128] (64 dims + ones col +
     junk), accumulated over 16 k-tiles into po [128, 512].  Row 64 = softmax
     denominator.  Normalization is folded into the drain: reciprocal of
     row 64, K=1 matmul broadcast to 64 rows, fused multiply into AOTU.
  5. Final projection per query tile: AOTU (normalized) @ w_out rows ->
     bf16 partial, DMA out.
"""

import numpy as np

import concourse.bass as bass
import concourse.bacc as bacc
import concourse.mybir as mybir
import concourse.tile as tile
from concourse.bass_utils import run_bass_kernel_spmd

N_CORES = 8
B, N, D = 4, 2048, 768
H_TOT, DH = 12, 64
HL = 6               # heads per core
NP = 3               # head pairs per core
DT = D // 128        # 6 partition tiles of the model dim
NT = N // 128        # 16 key tiles
QB = 512             # query block per phase
NPH = N // QB        # 4 phases per pair
VG = DH + 1          # 65
VW = HL * VG + 63    # 453: V cols + tail pad so last head's stationary is 128
CL = 3 * HL * DH     # 1152 local qkv columns (q 0:384 | k 384:768 | v 768:1152)

F32 = mybir.dt.float32
F32R = mybir.dt.float32r
BF16 = mybir.dt.bfloat16
I16 = mybir.dt.int16
ALU = mybir.AluOpType
AF = mybir.ActivationFunctionType

# Schraudolph exp on DVE: bf16 bits = int16(EXPA * s_raw + EXPB); folds the
# 0.125 attention scale into EXPA.  bcorr 6.0 minimizes max rel err (~3.5%).
EXPA = (128.0 / np.log(2.0)) * 0.125
EXPB = 127.0 * 128.0 - 6.0
# which k-tiles use the DVE approximation (rest: exact Exp on ScalarE).
# k=0..2 stay on ACT so the DVE can drain the previous phase's attn@V
# evacuations before its first exp of a new phase.
DVE_KS = frozenset({3, 5, 7, 9, 11, 13})


class Filler:
    """Round-robin generator pump: interleaves deferred instruction emission
    (attn@V of the previous phase, projections of the next pair) into the
    scores loop so the in-order PE queue always has 128x128-mode work
    between 64x128-mode score batches."""

    def __init__(self):
        self.gens = []

    def add(self, g):
        self.gens.append(g)

    def pull(self, n=1):
        for _ in range(n):
            while self.gens:
                try:
                    next(self.gens[0])
                    break
                except StopIteration:
                    self.gens.pop(0)
            if not self.gens:
                return

    def drain(self):
        while self.gens:
            self.pull()


def build():
    nc = bacc.Bacc("TRN2", target_bir_lowering=False, debug=False,
                   num_devices=N_CORES)

    xT_d = nc.dram_tensor("xT", [D, N], BF16, kind="ExternalInput")
    wqkv_d = nc.dram_tensor("wqkv", [D, CL], BF16, kind="ExternalInput")
    wout_d = nc.dram_tensor("wout", [HL * DH, D], BF16, kind="ExternalInput")
    out_d = nc.dram_tensor("out", [N, D], BF16, kind="ExternalOutput")

    with tile.TileContext(nc) as tc:
        with tc.tile_pool(name="persist", bufs=1) as pp, \
             tc.tile_pool(name="small", bufs=1) as smallp, \
             tc.tile_pool(name="projin", bufs=1) as projin, \
             tc.tile_pool(name="qk", bufs=1) as qkp, \
             tc.tile_pool(name="outs", bufs=4) as outsp:

            # ---- persistent tiles ----
            V = [pp.tile([128, VW], BF16, name=f"V{t}", tag=f"V{t}")
                 for t in range(NT)]
            AOTU = [pp.tile([128, N], BF16, name=f"AOTU{p}", tag=f"AOTU{p}")
                    for p in range(NP)]
            AOT = [pp.tile([128, N], BF16, name=f"AOT{p}", tag=f"AOT{p}")
                   for p in range(NP)]
            WO = [pp.tile([128, D], BF16, name=f"WO{i}", tag=f"WO{i}")
                  for i in range(DT // 2)]
            xT = [pp.tile([128, N], BF16, name=f"xT{d}", tag=f"xT{d}")
                  for d in range(DT)]
            # E1/E2: ones selector rows for broadcasting a [1, 512] reciprocal
            # to a head's 64 partition rows.  Padded to K=64 (rows 1..63 zero)
            # so the broadcast matmul runs in the same 64-row PE mode as the
            # scores and adds no extra mode transition.
            E1 = pp.tile([DH, 128], BF16, name="E1", tag="E1")
            E2 = pp.tile([DH, 128], BF16, name="E2", tag="E2")
            exw = smallp.tile([1, 128], F32, name="exw", tag="exw")

            # DMA issue on Sync serializes at ~600ns/descriptor: interleave
            # so pair-0's d=0 operands land first and Q proj starts early
            wv = [projin.tile([128, HL * DH], BF16, name=f"wv{d}",
                              tag=f"wv{d}") for d in range(DT)]
            wq0 = [projin.tile([128, 128], BF16, name=f"wq{d}", tag=f"wq{d}",
                               bufs=2) for d in range(DT)]
            wk0 = [projin.tile([128, 128], BF16, name=f"wk{d}", tag=f"wk{d}",
                               bufs=2) for d in range(DT)]
            for d in range(DT):
                nc.sync.dma_start(wq0[d][:],
                                  wqkv_d.ap()[d * 128:(d + 1) * 128, 0:128])
                nc.sync.dma_start(wk0[d][:],
                                  wqkv_d.ap()[d * 128:(d + 1) * 128,
                                              HL * DH:HL * DH + 128])
                nc.sync.dma_start(xT[d][:], xT_d.ap()[d * 128:(d + 1) * 128, :])
                nc.sync.dma_start(
                    wv[d][:],
                    wqkv_d.ap()[d * 128:(d + 1) * 128, 2 * HL * DH:CL])
            for i in range(DT // 2):
                nc.sync.dma_start(WO[i][:], wout_d.ap()[i * 128:(i + 1) * 128, :])

            RB0 = pp.tile([DH, QB], BF16, name="RB0", tag="RB0")
            RB1 = pp.tile([DH, QB], BF16, name="RB1", tag="RB1")
            nc.gpsimd.memset(E1[:], 0.0)
            nc.gpsimd.memset(E2[:], 0.0)
            nc.gpsimd.memset(E1[0:1, 0:DH], 1.0)
            nc.gpsimd.memset(E2[0:1, DH:128], 1.0)
            nc.gpsimd.memset(RB0[:], 0.0)
            nc.gpsimd.memset(RB1[:], 0.0)
            # warm the ACT exp table set during the DMA shadow
            nc.gpsimd.memset(exw[:], 0.0)
            nc.scalar.activation(exw[:], exw[:], AF.Exp, scale=0.125)
            for t in range(NT):
                ones = V[t][:, 0:HL * VG].rearrange(
                    "p (h c) -> p h c", c=VG)[:, :, DH:VG]
                nc.gpsimd.memset(ones, 1.0)
                nc.gpsimd.memset(V[t][:, HL * VG:], 0.0)

            qp_cur = [None]
            ktp_cur = [None]

            # ---- pair-0 Q^T/K^T projection, d-outer so matmuls start as
            # soon as each xT d-tile lands (startup is DMA-bound) ----
            with tc.tile_pool(name="psQK", bufs=1, space="PSUM") as psqk:
                Qp0 = qkp.tile([128, N], BF16, name="Qp", tag="Qp", bufs=2)
                KTp0 = qkp.tile([128, N], BF16, name="KTp", tag="KTp", bufs=2)
                for (w, dst) in ((wq0, Qp0), (wk0, KTp0)):
                    ps4 = [psqk.tile([128, QB], F32, name=f"pq{nb}",
                                     tag=f"pq{nb}") for nb in range(NPH)]
                    for d in range(DT):
                        for nb in range(NPH):
                            nc.tensor.matmul(
                                ps4[nb][:], w[d][:],
                                xT[d][:, nb * QB:(nb + 1) * QB],
                                start=(d == 0), stop=(d == DT - 1))
                    for nb in range(NPH):
                        eng = nc.vector if nb % 2 == 0 else nc.scalar
                        if nb % 2 == 0:
                            nc.vector.tensor_copy(
                                dst[:, nb * QB:(nb + 1) * QB], ps4[nb][:])
                        else:
                            nc.scalar.copy(
                                dst[:, nb * QB:(nb + 1) * QB], ps4[nb][:])
                qp_cur[0] = Qp0
                ktp_cur[0] = KTp0

            # ---- attention pools (8 PSUM banks exactly) ----
            with tc.tile_pool(name="pt", bufs=1) as ptp, \
                 tc.tile_pool(name="psS", bufs=2, space="PSUM") as psS, \
                 tc.tile_pool(name="psO", bufs=1, space="PSUM") as psO, \
                 tc.tile_pool(name="psA", bufs=2, space="PSUM") as psA:

                def projv_gen():
                    # V [2048, 6*65 layout]: xT stationary, wv moving
                    for t in range(NT):
                        ps = psA.tile([128, QB], F32, name="psA", tag="psA")
                        for d in range(DT):
                            nc.tensor.matmul(
                                ps[:, 0:HL * DH],
                                xT[d][:, t * 128:(t + 1) * 128], wv[d][:],
                                start=(d == 0), stop=(d == DT - 1))
                        dst = V[t][:, 0:HL * VG].rearrange(
                            "p (h c) -> p h c", c=VG)[:, :, 0:DH]
                        src = ps[:, 0:HL * DH].rearrange(
                            "p (h c) -> p h c", c=DH)
                        if t % 2 == 0:
                            nc.vector.tensor_copy(dst, src)
                        else:
                            nc.scalar.copy(dst, src)
                        yield

                def projqk_gen(p1):
                    wq = [projin.tile([128, 128], BF16, name=f"wq{d}",
                                      tag=f"wq{d}", bufs=2) for d in range(DT)]
                    wk = [projin.tile([128, 128], BF16, name=f"wk{d}",
                                      tag=f"wk{d}", bufs=2) for d in range(DT)]
                    for d in range(DT):
                        nc.sync.dma_start(
                            wq[d][:],
                            wqkv_d.ap()[d * 128:(d + 1) * 128,
                                        p1 * 128:(p1 + 1) * 128])
                        nc.sync.dma_start(
                            wk[d][:],
                            wqkv_d.ap()[d * 128:(d + 1) * 128,
                                        HL * DH + p1 * 128:
                                        HL * DH + (p1 + 1) * 128])
                    Qn = qkp.tile([128, N], BF16, name="Qp", tag="Qp", bufs=2)
                    Kn = qkp.tile([128, N], BF16, name="KTp", tag="KTp",
                                  bufs=2)
                    for (w, dst) in ((wq, Qn), (wk, Kn)):
                        for nb in range(NPH):
                            ps = psA.tile([128, QB], F32, name="psA",
                                          tag="psA")
                            for d in range(DT):
                                nc.tensor.matmul(
                                    ps[:], w[d][:],
                                    xT[d][:, nb * QB:(nb + 1) * QB],
                                    start=(d == 0), stop=(d == DT - 1))
                            nbs = slice(nb * QB, (nb + 1) * QB)
                            if nb % 2 == 0:
                                nc.vector.tensor_copy(dst[:, nbs], ps[:])
                            else:
                                nc.scalar.copy(dst[:, nbs], ps[:])
                            yield
                    qp_cur[0] = Qn
                    ktp_cur[0] = Kn

                def attnv_gen(p, qh, PTs):
                    # attn@V for both heads of pair p, query block qh; the
                    # drain folds softmax normalization into the PSUM copy
                    po0 = psO.tile([128, QB], F32, name="po0", tag="po0")
                    po1 = psO.tile([128, QB], F32, name="po1", tag="po1")
                    h0, h1 = 2 * p, 2 * p + 1
                    for k in range(NT):
                        nc.tensor.matmul(
                            po0[:], V[k][:, VG * h0:VG * h0 + 128],
                            PTs[k][:, 0:QB],
                            start=(k == 0), stop=(k == NT - 1))
                        nc.tensor.matmul(
                            po1[:], V[k][:, VG * h1:VG * h1 + 128],
                            PTs[k][:, QB:2 * QB],
                            start=(k == 0), stop=(k == NT - 1))
                        yield
                    qs = slice(qh * QB, (qh + 1) * QB)
                    # denominators to SBUF first: custom-DVE recip must not
                    # read PSUM
                    DD0 = smallp.tile([1, QB], F32, name="DD0", tag="DD0",
                                      bufs=2)
                    DD1 = smallp.tile([1, QB], F32, name="DD1", tag="DD1",
                                      bufs=2)
                    nc.vector.tensor_copy(DD0[:], po0[DH:DH + 1, :])
                    nc.vector.tensor_copy(DD1[:], po1[DH:DH + 1, :])
                    nc.scalar.copy(AOTU[p][0:DH, qs], po0[0:DH, :])
                    nc.vector.tensor_copy(AOTU[p][DH:128, qs], po1[0:DH, :])
                    Rf0 = smallp.tile([1, QB], F32, name="Rf0", tag="Rf0",
                                      bufs=2)
                    Rf1 = smallp.tile([1, QB], F32, name="Rf1", tag="Rf1",
                                      bufs=2)
                    nc.vector.reciprocal_approx_fast(Rf0[:], DD0[:])
                    nc.vector.reciprocal_approx_fast(Rf1[:], DD1[:])
                    nc.scalar.copy(RB0[0:1, :], Rf0[:])
                    nc.scalar.copy(RB1[0:1, :], Rf1[:])
                    yield
                    yield
                    yield
                    # broadcast recip rows to the pair's partitions (K=64,
                    # zero-padded selector rows -> same PE mode as scores).
                    # rb reuses po0's bank (frees once its copies drained) so
                    # the psA proj rotation is not coupled to this chain; the
                    # extra yields put ~2 score batches of PE work before the
                    # E-matmul so it never head-of-line blocks on the recips.
                    rb = psO.tile([128, QB], F32, name="rb", tag="po0")
                    nc.tensor.matmul(rb[:], E1[:], RB0[:],
                                     start=True, stop=False)
                    nc.tensor.matmul(rb[:], E2[:], RB1[:],
                                     start=False, stop=True)
                    yield
                    yield
                    nc.vector.tensor_mul(AOT[p][:, qs], AOTU[p][:, qs], rb[:])

                ogi = [0]

                def outproj_gen(qts):
                    # hoisted final-projection groups (AOT for these query
                    # tiles is complete); runs in pair-2 phases where the
                    # psA pool is otherwise idle
                    for qt in qts:
                        ot = outsp.tile([128, D], BF16, name="ot", tag="ot")
                        for (fo, fsz) in ((0, 512), (512, 256)):
                            ps = psA.tile([128, QB], F32, name="psA",
                                          tag="psA")
                            for i in range(NP):
                                nc.tensor.matmul(
                                    ps[:, :fsz],
                                    AOT[i][:, qt * 128:(qt + 1) * 128],
                                    WO[i][:, fo:fo + fsz],
                                    start=(i == 0), stop=(i == NP - 1))
                            if ogi[0] % 2 == 0:
                                nc.vector.tensor_copy(ot[:, fo:fo + fsz],
                                                      ps[:, :fsz])
                            else:
                                nc.scalar.copy(ot[:, fo:fo + fsz],
                                               ps[:, :fsz])
                            ogi[0] += 1
                            yield
                        nc.sync.dma_start(
                            out_d.ap()[qt * 128:(qt + 1) * 128, :], ot[:])

                def emit_phase(p, qh, fil, last):
                    Qp_, KTp_ = qp_cur[0], ktp_cur[0]
                    qs = slice(qh * QB, (qh + 1) * QB)
                    PTs = [ptp.tile([128, 2 * QB], BF16, name=f"PT{k}",
                                    tag=f"PT{k}", bufs=2) for k in range(NT)]
                    sg = [None]
                    for kk in range(0, NT, 2):
                        fil.pull(4)
                        for k in (kk, kk + 1):
                            ps = psS.tile([128, 2 * QB], F32, name="psS",
                                          tag="psS")
                            nc.tensor.matmul(
                                ps[:, 0:QB],
                                KTp_[0:DH, k * 128:(k + 1) * 128],
                                Qp_[0:DH, qs], start=True, stop=True)
                            nc.tensor.matmul(
                                ps[:, QB:2 * QB],
                                KTp_[DH:128, k * 128:(k + 1) * 128],
                                Qp_[DH:128, qs], start=True, stop=True)
                            if k in DVE_KS:
                                nc.vector.tensor_scalar(
                                    PTs[k][:].bitcast(I16), ps[:],
                                    EXPA, EXPB, ALU.mult, ALU.add)
                            else:
                                nc.scalar.activation(PTs[k][:], ps[:], AF.Exp,
                                                     scale=0.125)
                        if last and kk >= 2:
                            # self-interleave the final phase's attn@V with
                            # a 2 k-step lag so the tail is just the drain
                            if sg[0] is None:
                                sg[0] = attnv_gen(p, qh, PTs)
                            for _ in range(2):
                                try:
                                    next(sg[0])
                                except StopIteration:
                                    break
                    return PTs, sg[0]

                fil = Filler()
                for p in range(NP):
                    for qh in range(NPH):
                        last = (p == NP - 1 and qh == NPH - 1)
                        if p == 0 and qh == 0:
                            fil.add(projv_gen())
                        PTs, sg = emit_phase(p, qh, fil, last)
                        fil.drain()
                        if not last:
                            fil.add(attnv_gen(p, qh, PTs))
                            if qh == 2 and p + 1 < NP:
                                fil.add(projqk_gen(p + 1))
                            if p == NP - 1 and qh >= 1:
                                # hoist out-proj for query tiles whose AOT
                                # chunk was completed a full phase ago (so
                                # the fillers never stall on the normalize)
                                fil.add(outproj_gen(range(4 * (qh - 1),
                                                          4 * qh)))
                        else:
                            if sg is None:
                                sg = attnv_gen(p, qh, PTs)
                            for _ in sg:
                                pass

            # ---- final projection tail (qt 0..7 hoisted into pair-2
            # phases; attention PSUM pools released here) ----
            with tc.tile_pool(name="psC", bufs=4, space="PSUM") as psC:
                gi = 0
                for qt in range(8, NT):
                    ot = outsp.tile([128, D], BF16, name="ot", tag="ot")
                    for (fo, fsz) in ((0, 512), (512, 256)):
                        ps = psC.tile([128, 512], F32, name="psF", tag="psF")
                        for i in range(NP):
                            nc.tensor.matmul(
                                ps[:, :fsz],
                                AOT[i][:, qt * 128:(qt + 1) * 128],
                                WO[i][:, fo:fo + fsz],
                                start=(i == 0), stop=(i == NP - 1))
                        if gi % 2 == 0:
                            nc.vector.tensor_copy(ot[:, fo:fo + fsz],
                                                  ps[:, :fsz])
                        else:
                            nc.scalar.copy(ot[:, fo:fo + fsz], ps[:, :fsz])
                        gi += 1
                    nc.sync.dma_start(
                        out_d.ap()[qt * 128:(qt + 1) * 128, :], ot[:])

    nc.compile()
    return nc


_NC = None
_BOUT = None


def _get_nc():
    global _NC
    if _NC is None:
        _NC = build()
    return _NC


def make_in_maps(x, w_qkv, w_out, b_out):
    import ml_dtypes
    global _BOUT
    x = np.asarray(x, np.float32)
    w = np.asarray(w_qkv, np.float32)
    wo = np.asarray(w_out, np.float32)
    _BOUT = np.asarray(b_out, np.float32)
    inner = H_TOT * DH
    in_maps = []
    for c in range(N_CORES):
        b, hh = divmod(c, 2)
        cs = slice(hh * HL * DH, (hh + 1) * HL * DH)
        wloc = np.concatenate(
            [w[:, 0:inner][:, cs], w[:, inner:2 * inner][:, cs],
             w[:, 2 * inner:3 * inner][:, cs]], axis=1)
        in_maps.append({
            "xT": np.ascontiguousarray(x[b].T.astype(ml_dtypes.bfloat16)),
            "wqkv": np.ascontiguousarray(wloc.astype(ml_dtypes.bfloat16)),
            "wout": np.ascontiguousarray(wo[cs, :].astype(ml_dtypes.bfloat16)),
        })
    return in_maps


def run(in_maps, trace=False, **kw):
    return run_bass_kernel_spmd(_get_nc(), in_maps,
                                core_ids=list(range(N_CORES)),
                                trace=trace, **kw)


def assemble(results):
    out = np.empty((B, N, D), np.float32)
    for b in range(B):
        out[b] = (results[2 * b]["out"].astype(np.float32)
                  + results[2 * b + 1]["out"].astype(np.float32) + _BOUT)
    return out


def kernel(x, w_qkv, w_out, b_out):
    res = run(make_in_maps(x, w_qkv, w_out, b_out))
    return assemble(res.results)


# revision 49
# speedup vs baseline: 1.0195x; 1.0195x over previous
"""Multi-head attention (B=4, N=2048, D=768, H=12, Dh=64) on 8 TRN2 NeuronCores.

Sharding: core c -> batch b = c//2, head half hh = c%2 (6 heads each).
Each core computes its 6 heads over ALL 2048 queries x 2048 keys, so QKV
projection work is perfectly split (no duplicated K/V projection).  The final
projection is row-parallel over w_out: each core emits a bf16 partial
out = AO_local @ w_out[384 rows] and the host sums the two partials + bias.

Per-core graph (bf16 matmuls, f32 accumulation):
  1. Q^T/K^T pair panels [128=2x64 dims, 2048] projected per head pair.
  2. Scores per pair run ROW-TILED (64x128 PE mode): the two heads are
     independent K=64 matmuls on array row groups 0-63 / 64-127 executing
     concurrently -> 2x score throughput.  psS [128 keys, 1024] holds
     h0|h1 halves for one 512-query block.
  3. exp: 10 of 16 k-tiles exact Exp on ScalarE; 6 k-tiles a one-instruction
     Schraudolph approximation on VectorE (int16(23.083*s + 16250) bitcast
     to bf16 ~= exp(0.125 s), max rel err ~3.5%) -> both engines share the
     25M-element softmax which would otherwise bottleneck on ACT
     (~190us at 1 elem/cycle/lane).  End-to-end rel err ~9.2e-3 (gate 2e-2).
  4. attn@V per head: V stationary [128 keys, 128] (64 dims + ones col +
     junk), accumulated over 16 k-tiles into po [128, 512].  Row 64 = softmax
     denominator.  Normalization is folded into the drain: reciprocal of
     row 64, K=1 matmul broadcast to 64 rows, fused multiply into AOTU.
  5. Final projection per query tile: AOTU (normalized) @ w_out rows ->
     bf16 partial, DMA out.
"""

import numpy as np

import concourse.bass as bass
import concourse.bacc as bacc
import concourse.mybir as mybir
import concourse.tile as tile
from concourse.bass_utils import run_bass_kernel_spmd

N_CORES = 8
B, N, D = 4, 2048, 768
H_TOT, DH = 12, 64
HL = 6               # heads per core
NP = 3               # head pairs per core
DT = D // 128        # 6 partition tiles of the model dim
NT = N // 128        # 16 key tiles
QB = 512             # query block per phase
NPH = N // QB        # 4 phases per pair
VG = DH + 1          # 65
VW = HL * VG + 63    # 453: V cols + tail pad so last head's stationary is 128
CL = 3 * HL * DH     # 1152 local qkv columns (q 0:384 | k 384:768 | v 768:1152)

F32 = mybir.dt.float32
F32R = mybir.dt.float32r
BF16 = mybir.dt.bfloat16
I16 = mybir.dt.int16
ALU = mybir.AluOpType
AF = mybir.ActivationFunctionType

# Schraudolph exp on DVE: bf16 bits = int16(EXPA * s_raw + EXPB); folds the
# 0.125 attention scale into EXPA.  bcorr 6.0 minimizes max rel err (~3.5%).
EXPA = (128.0 / np.log(2.0)) * 0.125
EXPB = 127.0 * 128.0 - 6.0
# which k-tiles use the DVE approximation (rest: exact Exp on ScalarE).
# k=0..2 stay on ACT so the DVE can drain the previous phase's attn@V
# evacuations before its first exp of a new phase.
DVE_KS = frozenset({3, 5, 7, 9, 11, 13})


class Filler:
    """Round-robin generator pump: interleaves deferred instruction emission
    (attn@V of the previous phase, projections of the next pair) into the
    scores loop so the in-order PE queue always has 128x128-mode work
    between 64x128-mode score batches."""

    def __init__(self):
        self.gens = []

    def add(self, g):
        self.gens.append(g)

    def pull(self, n=1):
        for _ in range(n):
            while self.gens:
                try:
                    next(self.gens[0])
                    break
                except StopIteration:
                    self.gens.pop(0)
            if not self.gens:
                return

    def drain(self):
        while self.gens:
            self.pull()


def build():
    nc = bacc.Bacc("TRN2", target_bir_lowering=False, debug=False,
                   num_devices=N_CORES)

    xT_d = nc.dram_tensor("xT", [D, N], BF16, kind="ExternalInput")
    wqkv_d = nc.dram_tensor("wqkv", [D, CL], BF16, kind="ExternalInput")
    wout_d = nc.dram_tensor("wout", [HL * DH, D], BF16, kind="ExternalInput")
    out_d = nc.dram_tensor("out", [N, D], BF16, kind="ExternalOutput")

    with tile.TileContext(nc) as tc:
        with tc.tile_pool(name="persist", bufs=1) as pp, \
             tc.tile_pool(name="small", bufs=1) as smallp, \
             tc.tile_pool(name="projin", bufs=1) as projin, \
             tc.tile_pool(name="qk", bufs=1) as qkp, \
             tc.tile_pool(name="outs", bufs=4) as outsp:

            # ---- persistent tiles ----
            V = [pp.tile([128, VW], BF16, name=f"V{t}", tag=f"V{t}")
                 for t in range(NT)]
            AOTU = [pp.tile([128, N], BF16, name=f"AOTU{p}", tag=f"AOTU{p}")
                    for p in range(NP)]
            AOT = [pp.tile([128, N], BF16, name=f"AOT{p}", tag=f"AOT{p}")
                   for p in range(NP)]
            WO = [pp.tile([128, D], BF16, name=f"WO{i}", tag=f"WO{i}")
                  for i in range(DT // 2)]
            xT = [pp.tile([128, N], BF16, name=f"xT{d}", tag=f"xT{d}")
                  for d in range(DT)]
            # E1/E2: ones selector rows for broadcasting a [1, 512] reciprocal
            # to a head's 64 partition rows.  Padded to K=64 (rows 1..63 zero)
            # so the broadcast matmul runs in the same 64-row PE mode as the
            # scores and adds no extra mode transition.
            E1 = pp.tile([DH, 128], BF16, name="E1", tag="E1")
            E2 = pp.tile([DH, 128], BF16, name="E2", tag="E2")
            exw = smallp.tile([1, 128], F32, name="exw", tag="exw")

            # DMA issue on Sync serializes at ~600ns/descriptor: load each
            # d-tile's FULL local wqkv row-block as one resident tile (all
            # q/k/v panel weights for every pair), interleaved with xT so
            # pair-0's d=0 operands land first.  Kills the per-pair weight
            # DMAs entirely (and their mid-phase DMA-wait stalls).
            wall = [projin.tile([128, CL], BF16, name=f"wall{d}",
                                tag=f"wall{d}") for d in range(DT)]
            for d in range(DT):
                nc.sync.dma_start(xT[d][:], xT_d.ap()[d * 128:(d + 1) * 128, :])
                nc.sync.dma_start(wall[d][:],
                                  wqkv_d.ap()[d * 128:(d + 1) * 128, :])
            for i in range(DT // 2):
                nc.sync.dma_start(WO[i][:], wout_d.ap()[i * 128:(i + 1) * 128, :])

            RB0 = pp.tile([DH, QB], BF16, name="RB0", tag="RB0")
            RB1 = pp.tile([DH, QB], BF16, name="RB1", tag="RB1")
            nc.gpsimd.memset(E1[:], 0.0)
            nc.gpsimd.memset(E2[:], 0.0)
            nc.gpsimd.memset(E1[0:1, 0:DH], 1.0)
            nc.gpsimd.memset(E2[0:1, DH:128], 1.0)
            nc.gpsimd.memset(RB0[:], 0.0)
            nc.gpsimd.memset(RB1[:], 0.0)
            # warm the ACT exp table set during the DMA shadow
            nc.gpsimd.memset(exw[:], 0.0)
            nc.scalar.activation(exw[:], exw[:], AF.Exp, scale=0.125)
            for t in range(NT):
                ones = V[t][:, 0:HL * VG].rearrange(
                    "p (h c) -> p h c", c=VG)[:, :, DH:VG]
                nc.gpsimd.memset(ones, 1.0)
                nc.gpsimd.memset(V[t][:, HL * VG:], 0.0)

            qp_cur = [None]
            ktp_cur = [None]

            # ---- pair-0 Q^T/K^T projection, d-outer so matmuls start as
            # soon as each xT d-tile lands (startup is DMA-bound) ----
            with tc.tile_pool(name="psQK", bufs=1, space="PSUM") as psqk:
                Qp0 = qkp.tile([128, N], BF16, name="Qp", tag="Qp", bufs=2)
                KTp0 = qkp.tile([128, N], BF16, name="KTp", tag="KTp", bufs=2)
                for (co, dst) in ((0, Qp0), (HL * DH, KTp0)):
                    ps4 = [psqk.tile([128, QB], F32, name=f"pq{nb}",
                                     tag=f"pq{nb}") for nb in range(NPH)]
                    for d in range(DT):
                        for nb in range(NPH):
                            nc.tensor.matmul(
                                ps4[nb][:], wall[d][:, co:co + 128],
                                xT[d][:, nb * QB:(nb + 1) * QB],
                                start=(d == 0), stop=(d == DT - 1))
                    for nb in range(NPH):
                        eng = nc.vector if nb % 2 == 0 else nc.scalar
                        if nb % 2 == 0:
                            nc.vector.tensor_copy(
                                dst[:, nb * QB:(nb + 1) * QB], ps4[nb][:])
                        else:
                            nc.scalar.copy(
                                dst[:, nb * QB:(nb + 1) * QB], ps4[nb][:])
                qp_cur[0] = Qp0
                ktp_cur[0] = KTp0

            # ---- attention pools (8 PSUM banks exactly) ----
            with tc.tile_pool(name="pt", bufs=1) as ptp, \
                 tc.tile_pool(name="psS", bufs=2, space="PSUM") as psS, \
                 tc.tile_pool(name="psO", bufs=1, space="PSUM") as psO, \
                 tc.tile_pool(name="psA", bufs=2, space="PSUM") as psA:

                def projv_gen():
                    # V [2048, 6*65 layout]: xT stationary, wv moving
                    for t in range(NT):
                        ps = psA.tile([128, QB], F32, name="psA", tag="psA")
                        for d in range(DT):
                            nc.tensor.matmul(
                                ps[:, 0:HL * DH],
                                xT[d][:, t * 128:(t + 1) * 128],
                                wall[d][:, 2 * HL * DH:CL],
                                start=(d == 0), stop=(d == DT - 1))
                        dst = V[t][:, 0:HL * VG].rearrange(
                            "p (h c) -> p h c", c=VG)[:, :, 0:DH]
                        src = ps[:, 0:HL * DH].rearrange(
                            "p (h c) -> p h c", c=DH)
                        if t % 2 == 0:
                            nc.vector.tensor_copy(dst, src)
                        else:
                            nc.scalar.copy(dst, src)
                        yield

                def projqk_gen(p1):
                    Qn = qkp.tile([128, N], BF16, name="Qp", tag="Qp", bufs=2)
                    Kn = qkp.tile([128, N], BF16, name="KTp", tag="KTp",
                                  bufs=2)
                    for (co, dst) in ((p1 * 128, Qn),
                                      (HL * DH + p1 * 128, Kn)):
                        for nb in range(NPH):
                            ps = psA.tile([128, QB], F32, name="psA",
                                          tag="psA")
                            for d in range(DT):
                                nc.tensor.matmul(
                                    ps[:], wall[d][:, co:co + 128],
                                    xT[d][:, nb * QB:(nb + 1) * QB],
                                    start=(d == 0), stop=(d == DT - 1))
                            nbs = slice(nb * QB, (nb + 1) * QB)
                            if nb % 2 == 0:
                                nc.vector.tensor_copy(dst[:, nbs], ps[:])
                            else:
                                nc.scalar.copy(dst[:, nbs], ps[:])
                            yield
                    qp_cur[0] = Qn
                    ktp_cur[0] = Kn

                def attnv_gen(p, qh, PTs):
                    # attn@V for both heads of pair p, query block qh; the
                    # drain folds softmax normalization into the PSUM copy
                    po0 = psO.tile([128, QB], F32, name="po0", tag="po0")
                    po1 = psO.tile([128, QB], F32, name="po1", tag="po1")
                    h0, h1 = 2 * p, 2 * p + 1
                    for k in range(NT):
                        nc.tensor.matmul(
                            po0[:], V[k][:, VG * h0:VG * h0 + 128],
                            PTs[k][:, 0:QB],
                            start=(k == 0), stop=(k == NT - 1))
                        nc.tensor.matmul(
                            po1[:], V[k][:, VG * h1:VG * h1 + 128],
                            PTs[k][:, QB:2 * QB],
                            start=(k == 0), stop=(k == NT - 1))
                        yield
                    qs = slice(qh * QB, (qh + 1) * QB)
                    # denominators to SBUF first: custom-DVE recip must not
                    # read PSUM
                    DD0 = smallp.tile([1, QB], F32, name="DD0", tag="DD0",
                                      bufs=2)
                    DD1 = smallp.tile([1, QB], F32, name="DD1", tag="DD1",
                                      bufs=2)
                    nc.vector.tensor_copy(DD0[:], po0[DH:DH + 1, :])
                    nc.vector.tensor_copy(DD1[:], po1[DH:DH + 1, :])
                    nc.scalar.copy(AOTU[p][0:DH, qs], po0[0:DH, :])
                    nc.vector.tensor_copy(AOTU[p][DH:128, qs], po1[0:DH, :])
                    Rf0 = smallp.tile([1, QB], F32, name="Rf0", tag="Rf0",
                                      bufs=2)
                    Rf1 = smallp.tile([1, QB], F32, name="Rf1", tag="Rf1",
                                      bufs=2)
                    nc.vector.reciprocal_approx_fast(Rf0[:], DD0[:])
                    nc.vector.reciprocal_approx_fast(Rf1[:], DD1[:])
                    nc.scalar.copy(RB0[0:1, :], Rf0[:])
                    nc.scalar.copy(RB1[0:1, :], Rf1[:])
                    yield
                    yield
                    yield
                    # broadcast recip rows to the pair's partitions (K=64,
                    # zero-padded selector rows -> same PE mode as scores).
                    # rb reuses po0's bank (frees once its copies drained) so
                    # the psA proj rotation is not coupled to this chain; the
                    # extra yields put ~2 score batches of PE work before the
                    # E-matmul so it never head-of-line blocks on the recips.
                    rb = psO.tile([128, QB], F32, name="rb", tag="po0")
                    nc.tensor.matmul(rb[:], E1[:], RB0[:],
                                     start=True, stop=False)
                    nc.tensor.matmul(rb[:], E2[:], RB1[:],
                                     start=False, stop=True)
                    yield
                    yield
                    nc.vector.tensor_mul(AOT[p][:, qs], AOTU[p][:, qs], rb[:])

                ogi = [0]

                def outproj_gen(qts):
                    # hoisted final-projection groups (AOT for these query
                    # tiles is complete); runs in pair-2 phases where the
                    # psA pool is otherwise idle
                    for qt in qts:
                        ot = outsp.tile([128, D], BF16, name="ot", tag="ot")
                        for (fo, fsz) in ((0, 512), (512, 256)):
                            ps = psA.tile([128, QB], F32, name="psA",
                                          tag="psA")
                            for i in range(NP):
                                nc.tensor.matmul(
                                    ps[:, :fsz],
                                    AOT[i][:, qt * 128:(qt + 1) * 128],
                                    WO[i][:, fo:fo + fsz],
                                    start=(i == 0), stop=(i == NP - 1))
                            if ogi[0] % 2 == 0:
                                nc.vector.tensor_copy(ot[:, fo:fo + fsz],
                                                      ps[:, :fsz])
                            else:
                                nc.scalar.copy(ot[:, fo:fo + fsz],
                                               ps[:, :fsz])
                            ogi[0] += 1
                            yield
                        nc.sync.dma_start(
                            out_d.ap()[qt * 128:(qt + 1) * 128, :], ot[:])

                def emit_phase(p, qh, fil, last):
                    Qp_, KTp_ = qp_cur[0], ktp_cur[0]
                    qs = slice(qh * QB, (qh + 1) * QB)
                    PTs = [ptp.tile([128, 2 * QB], BF16, name=f"PT{k}",
                                    tag=f"PT{k}", bufs=2) for k in range(NT)]
                    sg = [None]
                    for kk in range(0, NT, 2):
                        fil.pull(4)
                        for k in (kk, kk + 1):
                            ps = psS.tile([128, 2 * QB], F32, name="psS",
                                          tag="psS")
                            nc.tensor.matmul(
                                ps[:, 0:QB],
                                KTp_[0:DH, k * 128:(k + 1) * 128],
                                Qp_[0:DH, qs], start=True, stop=True)
                            nc.tensor.matmul(
                                ps[:, QB:2 * QB],
                                KTp_[DH:128, k * 128:(k + 1) * 128],
                                Qp_[DH:128, qs], start=True, stop=True)
                            if k in DVE_KS:
                                nc.vector.tensor_scalar(
                                    PTs[k][:].bitcast(I16), ps[:],
                                    EXPA, EXPB, ALU.mult, ALU.add)
                            else:
                                nc.scalar.activation(PTs[k][:], ps[:], AF.Exp,
                                                     scale=0.125)
                        if last and kk >= 2:
                            # self-interleave the final phase's attn@V with
                            # a 2 k-step lag so the tail is just the drain
                            if sg[0] is None:
                                sg[0] = attnv_gen(p, qh, PTs)
                            for _ in range(2):
                                try:
                                    next(sg[0])
                                except StopIteration:
                                    break
                    return PTs, sg[0]

                fil = Filler()
                for p in range(NP):
                    for qh in range(NPH):
                        last = (p == NP - 1 and qh == NPH - 1)
                        if p == 0 and qh == 0:
                            fil.add(projv_gen())
                        PTs, sg = emit_phase(p, qh, fil, last)
                        fil.drain()
                        if not last:
                            fil.add(attnv_gen(p, qh, PTs))
                            if qh == 2 and p + 1 < NP:
                                fil.add(projqk_gen(p + 1))
                            if p == NP - 1 and qh >= 1:
                                # hoist out-proj for query tiles whose AOT
                                # chunk was completed a full phase ago (so
                                # the fillers never stall on the normalize)
                                fil.add(outproj_gen(range(4 * (qh - 1),
                                                          4 * qh)))
                        else:
                            if sg is None:
                                sg = attnv_gen(p, qh, PTs)
                            for _ in sg:
                                pass

            # ---- final projection tail (qt 0..7 hoisted into pair-2
            # phases; attention PSUM pools released here) ----
            with tc.tile_pool(name="psC", bufs=4, space="PSUM") as psC:
                gi = 0
                for qt in range(8, NT):
                    ot = outsp.tile([128, D], BF16, name="ot", tag="ot")
                    for (fo, fsz) in ((0, 512), (512, 256)):
                        ps = psC.tile([128, 512], F32, name="psF", tag="psF")
                        for i in range(NP):
                            nc.tensor.matmul(
                                ps[:, :fsz],
                                AOT[i][:, qt * 128:(qt + 1) * 128],
                                WO[i][:, fo:fo + fsz],
                                start=(i == 0), stop=(i == NP - 1))
                        if gi % 2 == 0:
                            nc.vector.tensor_copy(ot[:, fo:fo + fsz],
                                                  ps[:, :fsz])
                        else:
                            nc.scalar.copy(ot[:, fo:fo + fsz], ps[:, :fsz])
                        gi += 1
                    nc.sync.dma_start(
                        out_d.ap()[qt * 128:(qt + 1) * 128, :], ot[:])

    nc.compile()
    return nc


_NC = None
_BOUT = None


def _get_nc():
    global _NC
    if _NC is None:
        _NC = build()
    return _NC


def make_in_maps(x, w_qkv, w_out, b_out):
    import ml_dtypes
    global _BOUT
    x = np.asarray(x, np.float32)
    w = np.asarray(w_qkv, np.float32)
    wo = np.asarray(w_out, np.float32)
    _BOUT = np.asarray(b_out, np.float32)
    inner = H_TOT * DH
    in_maps = []
    for c in range(N_CORES):
        b, hh = divmod(c, 2)
        cs = slice(hh * HL * DH, (hh + 1) * HL * DH)
        wloc = np.concatenate(
            [w[:, 0:inner][:, cs], w[:, inner:2 * inner][:, cs],
             w[:, 2 * inner:3 * inner][:, cs]], axis=1)
        in_maps.append({
            "xT": np.ascontiguousarray(x[b].T.astype(ml_dtypes.bfloat16)),
            "wqkv": np.ascontiguousarray(wloc.astype(ml_dtypes.bfloat16)),
            "wout": np.ascontiguousarray(wo[cs, :].astype(ml_dtypes.bfloat16)),
        })
    return in_maps


def run(in_maps, trace=False, **kw):
    return run_bass_kernel_spmd(_get_nc(), in_maps,
                                core_ids=list(range(N_CORES)),
                                trace=trace, **kw)


def assemble(results):
    out = np.empty((B, N, D), np.float32)
    for b in range(B):
        out[b] = (results[2 * b]["out"].astype(np.float32)
                  + results[2 * b + 1]["out"].astype(np.float32) + _BOUT)
    return out


def kernel(x, w_qkv, w_out, b_out):
    res = run(make_in_maps(x, w_qkv, w_out, b_out))
    return assemble(res.results)
